# revision 1
# baseline (speedup 1.0000x reference)
"""Trainium2 Bass kernel for nn_DecoderLayer (self-attn + cross-attn + FFN).

Sharding: 8 cores, no collectives. Core c handles batch b=c//2, query-row
half r=c%2 (512 of 1024 rows). All per-core differences flow through input
data (host slices/transposes/permutes), so one SPMD NEFF serves all cores.

On-device layout is feature-major ("transposed"): activations live as
[channels(partitions), tokens(free)]. Weights are host-pre-transposed to
[in_ch, out_ch] and cast to bf16. Matmul operands are bf16 (fp32 PSUM
accumulation); the residual stream stays fp32.

x is passed column-PERMUTED so this core's 512 query tokens are always
columns 0:512 -- attention is permutation-equivariant over key positions,
and the host un-permutes the returned attention-weight rows.

Key PE-efficiency devices (HAM clock gate wants a dense matmul stream):
- V is stored in a 65-column-stride layout with a 16.0-valued extra column
  per head, so each attn@V matmul also accumulates that head's softmax
  denominator (x16) into PSUM row 64 -- no separate denominator matmuls.
- 1/denominator via the fast approximate DVE reciprocal; LN's 1/sqrt(var)
  is a single scalar-engine Abs_reciprocal_sqrt with the epsilon folded
  into the activation bias.
- Scores accumulate into 2-bank PSUM tiles so exp runs 1024 wide.
- Elementwise work is split DVE/Pool (prob-mean accumulate, LN mean-sub,
  V bias-adds on Pool) to keep both vector engines off the critical path.
- Cross-attention K projections fill the PE during self-attention pairs;
  cross V projections fill the self out-proj/LN2/Q2 window and the early
  cross-attention pairs, so the PE never idles long enough to re-throttle.
"""

from collections import deque

import ml_dtypes
import numpy as np

import concourse.bacc as bacc
import concourse.mybir as mybir
import concourse.tile as tile
from concourse.bass_utils import run_bass_kernel_spmd

F32 = mybir.dt.float32
BF16 = mybir.dt.bfloat16
AF = mybir.ActivationFunctionType
OP = mybir.AluOpType

P = 128
D = 1024
DFF = 4096
H = 16
B = 4
L = 1024          # full sequence (keys/values)
LQ = 512          # per-core query tokens
NC = D // P       # 8 channel chunks
NF = DFF // P     # 32 ff chunks
NSC = L // P      # 8 key-position chunks
VST = 65          # per-head V stride (64 channels + 1 denominator column)
EPS = 1e-5


def _build():
    nc = bacc.Bacc("TRN2", target_bir_lowering=False)

    xT16 = nc.dram_tensor("xT16", [D, L], BF16, kind="ExternalInput")    # permuted x[b].T bf16
    xaT16 = nc.dram_tensor("xaT16", [D, L], BF16, kind="ExternalInput")  # xa[b].T bf16
    w_sa = nc.dram_tensor("w_sa", [D, 3 * D], BF16, kind="ExternalInput")
    b_sa = nc.dram_tensor("b_sa", [3 * D], F32, kind="ExternalInput")
    wo_sa = nc.dram_tensor("wo_sa", [D, D], BF16, kind="ExternalInput")
    bo_sa = nc.dram_tensor("bo_sa", [D], F32, kind="ExternalInput")
    w_ca = nc.dram_tensor("w_ca", [D, 3 * D], BF16, kind="ExternalInput")
    b_ca = nc.dram_tensor("b_ca", [3 * D], F32, kind="ExternalInput")
    wo_ca = nc.dram_tensor("wo_ca", [D, D], BF16, kind="ExternalInput")
    bo_ca = nc.dram_tensor("bo_ca", [D], F32, kind="ExternalInput")
    w1 = nc.dram_tensor("w1", [D, DFF], BF16, kind="ExternalInput")
    b1 = nc.dram_tensor("b1", [DFF], F32, kind="ExternalInput")
    w2 = nc.dram_tensor("w2", [DFF, D], BF16, kind="ExternalInput")
    b2 = nc.dram_tensor("b2", [D], F32, kind="ExternalInput")
    ln_w = nc.dram_tensor("ln_w", [3, D], F32, kind="ExternalInput")
    ln_b = nc.dram_tensor("ln_b", [3, D], F32, kind="ExternalInput")

    xoutT = nc.dram_tensor("xoutT", [D, LQ], F32, kind="ExternalOutput")
    selfwT = nc.dram_tensor("selfwT", [L, LQ], F32, kind="ExternalOutput")
    crosswT = nc.dram_tensor("crosswT", [L, LQ], F32, kind="ExternalOutput")

    with tile.TileContext(nc) as tc:
        _emit(nc, tc, locals())
    nc.compile()
    return nc


def _emit(nc, tc, t):
    import contextlib
    import os
    DBG = os.environ.get("KBDBG", "")
    ctx = contextlib.ExitStack()
    with ctx:
        const = ctx.enter_context(tc.tile_pool(name="const", bufs=1))
        big = ctx.enter_context(tc.tile_pool(name="big", bufs=1))
        wproj = ctx.enter_context(tc.tile_pool(name="wproj", bufs=8))
        wkv2 = ctx.enter_context(tc.tile_pool(name="wkv2", bufs=8))
        sm = ctx.enter_context(tc.tile_pool(name="sm", bufs=3))     # [1,512] rows
        rep = ctx.enter_context(tc.tile_pool(name="rep", bufs=2))   # broadcast tiles
        expp = ctx.enter_context(tc.tile_pool(name="expp", bufs=9))  # [P,2,512] prob tiles
        outp = ctx.enter_context(tc.tile_pool(name="outp", bufs=3))  # transient tiles
        ps = ctx.enter_context(tc.tile_pool(name="ps", bufs=2, space="PSUM"))
        ps_s = ctx.enter_context(tc.tile_pool(name="ps_s", bufs=2, space="PSUM"))
        ps_av = ctx.enter_context(tc.tile_pool(name="ps_av", bufs=2, space="PSUM"))

        # ---- constants ----
        lnw_sb = const.tile([P, 3, NC], F32, name="lnw_sb")
        nc.sync.dma_start(out=lnw_sb, in_=t["ln_w"].rearrange("k (o p) -> p k o", p=P))
        lnb_sb = const.tile([P, 3, NC], F32, name="lnb_sb")
        nc.sync.dma_start(out=lnb_sb, in_=t["ln_b"].rearrange("k (o p) -> p k o", p=P))
        bqk_sa = const.tile([P, 16], F32, name="bqk_sa")
        nc.sync.dma_start(out=bqk_sa, in_=t["b_sa"][: 2 * D].rearrange("(o p) -> p o", p=P))
        bqk_ca = const.tile([P, 16], F32, name="bqk_ca")
        nc.sync.dma_start(out=bqk_ca, in_=t["b_ca"][: 2 * D].rearrange("(o p) -> p o", p=P))
        bo_sa_sb = const.tile([P, NC], F32, name="bo_sa_sb")
        nc.sync.dma_start(out=bo_sa_sb, in_=t["bo_sa"].rearrange("(o p) -> p o", p=P))
        bo_ca_sb = const.tile([P, NC], F32, name="bo_ca_sb")
        nc.sync.dma_start(out=bo_ca_sb, in_=t["bo_ca"].rearrange("(o p) -> p o", p=P))
        b1_sb = const.tile([P, NF], F32, name="b1_sb")
        nc.sync.dma_start(out=b1_sb, in_=t["b1"].rearrange("(o p) -> p o", p=P))
        b2_sb = const.tile([P, NC], F32, name="b2_sb")
        nc.sync.dma_start(out=b2_sb, in_=t["b2"].rearrange("(o p) -> p o", p=P))
        # v-bias rows replicated across partitions (staged via transient rows)
        bv_reps = {}
        for key in ("sa", "ca"):
            bv_rep = const.tile([P, D], BF16, name=f"bv_{key}_rep")
            for j in range(2):
                row = sm.tile([1, 512], F32, name=f"bv_{key}_row", tag="row")
                nc.sync.dma_start(
                    out=row, in_=t[f"b_{key}"][None, 2 * D + 512 * j: 2 * D + 512 * j + 512])
                row16 = sm.tile([1, 512], BF16, name=f"bv_{key}_row16", tag="row16", bufs=2)
                nc.vector.tensor_copy(row16, row)
                nc.gpsimd.partition_broadcast(bv_rep[:, 512 * j: 512 * j + 512], row16)
            bv_reps[key] = bv_rep
        ones_sb = const.tile([P, 1], BF16, name="ones_sb")
        nc.vector.memset(ones_sb, 1.0)
        eps_col = const.tile([1, 1], F32, name="eps_col")
        nc.vector.memset(eps_col, EPS)

        def ln_stats_tile(name):
            """One PSUM bank holding sum (row 0) and sum-sq (row 32)."""
            return ps_av.tile([33, 512], F32, name=name, tag="av")

        def ln_stats_chunk(st, x_chunk, o, name):
            xb = outp.tile([P, 512], BF16, name=name + "_xb", tag="lnt")
            nc.vector.tensor_copy(xb, x_chunk)
            sq = outp.tile([P, 512], BF16, name=name + "_sq", tag="lnt")
            nc.scalar.activation(sq, x_chunk, AF.Square)
            nc.tensor.matmul(st[0:1, :], ones_sb, xb,
                             start=(o == 0), stop=(o == NC - 1),
                             skip_group_check=True)
            nc.tensor.matmul(st[32:33, :], ones_sb, sq,
                             start=(o == 0), stop=(o == NC - 1),
                             skip_group_check=True)

        def ln_reduce(st, name):
            """stats psum -> (mean_rep, rsq_rep) f32 broadcast tiles."""
            mean = sm.tile([1, 512], F32, name=name + "_mean", tag="row")
            nc.vector.tensor_scalar_mul(mean, st[0:1, :], 1.0 / D)
            mean_rep = rep.tile([P, 512], F32, name=name + "_mrep", tag="rep")
            nc.gpsimd.partition_broadcast(mean_rep, mean)
            m2 = sm.tile([1, 512], F32, name=name + "_m2", tag="row")
            nc.vector.tensor_tensor(m2, mean, mean, OP.mult)
            var = sm.tile([1, 512], F32, name=name + "_var", tag="row")
            nc.vector.scalar_tensor_tensor(var, st[32:33, :], 1.0 / D, m2,
                                           OP.mult, OP.subtract)
            rsq = sm.tile([1, 512], F32, name=name + "_rsq", tag="row")
            nc.scalar.activation(rsq, var, AF.Abs_reciprocal_sqrt, bias=eps_col)
            rsq_rep = rep.tile([P, 512], F32, name=name + "_rrep", tag="rep")
            nc.gpsimd.partition_broadcast(rsq_rep, rsq)
            return mean_rep, rsq_rep

        def ln_finish_chunk(x_chunk, mean_rep, rsq_rep, ln_idx, o, out_b_chunk,
                            name, out_f_chunk=None):
            u = outp.tile([P, 512], F32, name=name + "_u", tag="lnu", bufs=2)
            nc.vector.tensor_tensor(u, x_chunk, mean_rep, OP.subtract)
            v = outp.tile([P, 512], F32, name=name + "_v", tag="lnv", bufs=2)
            nc.vector.scalar_tensor_tensor(
                v, u, lnw_sb[:, ln_idx, o: o + 1], rsq_rep, OP.mult, OP.mult)
            if out_f_chunk is not None:
                nc.scalar.activation(out_f_chunk, v, AF.Identity,
                                     bias=lnb_sb[:, ln_idx, o: o + 1])
                nc.vector.tensor_copy(out_b_chunk, out_f_chunk)
            else:
                nc.scalar.activation(out_b_chunk, v, AF.Identity,
                                     bias=lnb_sb[:, ln_idx, o: o + 1])

        def layer_norm_full(x16, ln_idx, name, tag, f32_tag, fillers):
            """LN over [P, NC, L] from the bf16 x copy. Returns
            (bf16 [P,NC,L], f32 [P,NC,LQ]). kv half (j=1) runs first; its
            serial finish chain overlaps the query half's stats matmuls.
            The query-half finish interleaves filler closures -- legal only
            there because the fillers overwrite x16's buffer."""
            out_b = big.tile([P, NC, L], BF16, name=name + "_ln", tag=tag)
            out_f = big.tile([P, NC, LQ], F32, name=name + "_lnf", tag=f32_tag)

            def stats(j):
                sl = slice(512 * j, 512 * j + 512)
                st = ps_s.tile([33, 512], F32, name=name + "_st", tag="sc")
                for o in range(NC):
                    # stats straight off the bf16 copy -- no staging copies
                    sq = outp.tile([P, 512], BF16, name=name + "_sq", tag="lnt")
                    nc.scalar.activation(sq, x16[:, o, sl], AF.Square)
                    nc.tensor.matmul(st[0:1, :], ones_sb, x16[:, o, sl],
                                     start=(o == 0), stop=(o == NC - 1),
                                     skip_group_check=True)
                    nc.tensor.matmul(st[32:33, :], ones_sb, sq,
                                     start=(o == 0), stop=(o == NC - 1),
                                     skip_group_check=True)
                return st

            st1 = stats(1)
            mean_rep, rsq_rep = ln_reduce(st1, name)
            st0 = stats(0)  # query-half stats matmuls fill the j=1 finish
            for o in range(NC):
                ln_finish_chunk(x16[:, o, 512:1024], mean_rep, rsq_rep, ln_idx,
                                o, out_b[:, o, 512:1024], name)
            mean_rep, rsq_rep = ln_reduce(st0, name)
            for o in range(NC):
                ln_finish_chunk(x16[:, o, 0:512], mean_rep, rsq_rep, ln_idx,
                                o, out_b[:, o, 0:512], name,
                                out_f_chunk=out_f[:, o, :])
                if fillers:
                    fillers.popleft()()
            return out_b, out_f

        def ln_make_split(ln_idx, name, tag, fillers=None, warm=False):
            """Split LN over [P, NC, 512]: returns (stats_chunk, finish).
            warm=True emits a tiny dummy matmul per chunk (keyed to the chunk
            output) purely to keep the HAM clock gate open through the
            otherwise matmul-free finish chain."""
            st_box = {}

            def stats_chunk(x_chunk, o):
                if "st" not in st_box:
                    st_box["st"] = ln_stats_tile(name + "_st")
                ln_stats_chunk(st_box["st"], x_chunk, o, name)

            def finish(x_sb):
                out_b = big.tile([P, NC, 512], BF16, name=name + "_ln", tag=tag)
                mean_rep, rsq_rep = ln_reduce(st_box["st"], name)
                warm_ps = (ps.tile([1, 64], F32, name=name + "_warm",
                                   tag="proj") if warm else None)
                for o in range(NC):
                    ln_finish_chunk(x_sb[:, o, :], mean_rep, rsq_rep, ln_idx, o,
                                    out_b[:, o, :], name)
                    if fillers:
                        fillers.popleft()()
                    elif warm:
                        nc.tensor.matmul(warm_ps, ones_sb, out_b[:, o, 0:64],
                                         start=True, stop=True,
                                         skip_group_check=True)
                return out_b

            return stats_chunk, finish

        def stream_w(pool, dram, k, lo, hi, name):
            w_t = pool.tile([P, hi - lo], BF16, name=name, tag="wp")
            nc.sync.dma_start(out=w_t, in_=dram[P * k: P * k + P, lo:hi])
            return w_t

        def q_proj(xq_b, w_dram, bqk, tagpfx, fillers=None):
            qT = big.tile([P, NC, LQ], BF16, name=tagpfx + "qT", tag="qT")
            wch = [stream_w(wproj, w_dram, k, 0, D, tagpfx + "wq") for k in range(NC)]
            for m in range(NC):
                acc = ps.tile([P, 512], F32, name=tagpfx + "qps", tag="proj")
                for k in range(NC):
                    nc.tensor.matmul(acc, wch[k][:, 128 * m: 128 * m + 128],
                                     xq_b[:, k, :], start=(k == 0), stop=(k == NC - 1))
                nc.scalar.activation(qT[:, m, :], acc, AF.Identity, bias=bqk[:, m: m + 1])
                if fillers and m % 2 == 1:
                    fillers.popleft()()
            return qT

        def k_proj_iter(wch, xkv_b, bqk, kT, m, j):
            acc = ps.tile([P, 512], F32, name="kps", tag="proj")
            for k in range(NC):
                nc.tensor.matmul(
                    acc, wch[k][:, 128 * m: 128 * m + 128],
                    xkv_b[:, k, 512 * j: 512 * j + 512],
                    start=(k == 0), stop=(k == NC - 1))
            nc.scalar.activation(kT[:, m, 512 * j: 512 * j + 512], acc,
                                 AF.Identity, bias=bqk[:, 8 + m: 9 + m])

        def vslice(vnat, m, h0, nh):
            """[P, nh, 64] view of V channels for heads h0..h0+nh at key-chunk m."""
            base = VST * h0
            return vnat[:, m, base: base + VST * nh].rearrange(
                "p (h c) -> p h c", c=VST)[:, :, 0:64]

        def v_proj_iter(wch, xkv_b, bv_rep, vnat, m, h0, nh, eng=None):
            """V proj for heads h0..h0+nh (nh*64 output channels), key-chunk m."""
            acc = ps.tile([P, 64 * nh], F32, name="vps", tag="proj")
            for k in range(NC):
                nc.tensor.matmul(
                    acc, xkv_b[:, k, 128 * m: 128 * m + 128],
                    wch[k][:, 64 * h0: 64 * h0 + 64 * nh],
                    start=(k == 0), stop=(k == NC - 1))
            (eng or nc.vector).tensor_tensor(
                vslice(vnat, m, h0, nh),
                acc.rearrange("p (h c) -> p h c", c=64),
                bv_rep[:, 64 * h0: 64 * h0 + 64 * nh].rearrange(
                    "p (h c) -> p h c", c=64),
                OP.add)

        def v_ones(vnat, name):
            # 16.0 in every head's denominator column: folds the 1/H head-mean
            # into the reciprocal (out-proj weights are host-scaled by 16).
            view = vnat.rearrange("p s (h c) -> p s h c", c=VST)[:, :, :, 64]
            nc.vector.memset(view, 16.0)

        def kv_proj(xkv_b, w_dram, bqk, bv_rep, kT, vnat, tagpfx):
            wch = [stream_w(wproj, w_dram, k, D, 2 * D, tagpfx + "wk")
                   for k in range(NC)]
            for m in range(NC):
                for j in range(2):
                    k_proj_iter(wch, xkv_b, bqk, kT, m, j)
            wch = [stream_w(wproj, w_dram, k, 2 * D, 3 * D, tagpfx + "wv")
                   for k in range(NC)]
            for m in range(NSC):
                for h0 in range(0, H, 8):
                    v_proj_iter(wch, xkv_b, bv_rep, vnat, m, h0, 8)

        def attention(qT, kT, vnat, swacc, tagpfx, points):
            """Returns aoT [P, NC, LQ] bf16 (normalized attn out, transposed).
            Accumulates head-mean probs into swacc [P, NSC, LQ] f32.
            points: 16 lists of filler closures (independent PE work), run at
            point 2g (after pair g's scores) and 2g+1 (between the AV heads) --
            schedule closures so producers precede their consuming pair."""
            aoT = big.tile([P, NC, LQ], BF16, name=tagpfx + "aoT", tag="aoT")
            deferred = []
            npairs = H // 2

            def pop_fillers(pt):
                for f in points[pt]:
                    f()

            for g in range(npairs):
                pair_exps = []   # per head: list of 4 [P,2,512] bf16 tiles
                pair_reps = []   # per head: [P,512] bf16 reciprocal rep
                pair_pav = []
                # scores + exp for both heads
                for hh in range(2):
                    base = 64 * hh
                    exps = []
                    for scp in range(4):
                        pss = ps_s.tile([P, 2, 512], F32, name=tagpfx + "pss", tag="sc")
                        for half in range(2):
                            sc = 2 * scp + half
                            nc.tensor.matmul(
                                pss[:, half, :],
                                kT[base: base + 64, g, 128 * sc: 128 * sc + 128],
                                qT[base: base + 64, g, :],
                                start=True, stop=True, skip_group_check=True)
                        e = expp.tile([P, 2, 512], BF16, name=tagpfx + "exp", tag="exp")
                        nc.scalar.activation(e, pss, AF.Exp, scale=0.125)
                        exps.append(e)
                    pair_exps.append(exps)
                # independent PE work bridging the exp (Act) latency
                pop_fillers(2 * g)
                # attn@V with fused denominator (row 64), both heads
                for hh in range(2):
                    h = 2 * g + hh
                    pav = ps_av.tile([65, 512], F32, name=tagpfx + "pav", tag="av")
                    for sc in range(NSC):
                        nc.tensor.matmul(
                            pav, vnat[:, sc, VST * h: VST * h + VST],
                            pair_exps[hh][sc // 2][:, sc % 2, :],
                            start=(sc == 0), stop=(sc == NSC - 1),
                            skip_group_check=True)
                    pair_pav.append(pav)
                    # custom-DVE recip mis-reads PSUM inputs: stage via SBUF
                    den = sm.tile([1, 512], F32, name=tagpfx + "den", tag="row")
                    nc.scalar.activation(den, pav[64:65, :], AF.Identity)
                    rec = sm.tile([1, 512], F32, name=tagpfx + "rec",
                                  tag="row")
                    nc.vector.reciprocal_approx_fast(rec, den)
                    rec16 = sm.tile([1, 512], BF16, name=tagpfx + "rec16",
                                    tag="row16", bufs=2)
                    nc.gpsimd.tensor_copy(rec16, rec)
                    rec_rep = rep.tile([P, 512], BF16, name=tagpfx + "rrep",
                                       tag="rep16", bufs=2)
                    nc.gpsimd.partition_broadcast(rec_rep, rec16)
                    pair_reps.append(rec_rep)
                    if hh == 0:
                        pop_fillers(2 * g + 1)
                # normalized attention out: head0 direct, head1 via DMA
                # partition-shift (PSUM rows 0:64 -> aoT partitions 64:128)
                nc.vector.tensor_tensor(
                    aoT[0:64, g, :], pair_pav[0][0:64, :],
                    pair_reps[0][0:64, :], OP.mult)
                tmp = outp.tile([64, 512], BF16, name=tagpfx + "aotmp", tag="aotmp",
                                bufs=2)
                nc.vector.tensor_tensor(
                    tmp, pair_pav[1][0:64, :], pair_reps[1][0:64, :], OP.mult)
                nc.sync.dma_start(out=aoT[64:128, g, :], in_=tmp)

                def swacc_work(g=g, pair_exps=pair_exps, pair_reps=pair_reps):
                    # scale both heads, pair-add, accumulate -- all DVE bf16.
                    for scp in range(4):
                        scl0 = outp.tile([P, 2, 512], BF16, name=tagpfx + "s0",
                                         tag="scl")
                        scl1 = outp.tile([P, 2, 512], BF16, name=tagpfx + "s1",
                                         tag="scl")
                        for half in range(2):
                            nc.vector.tensor_tensor(
                                scl0[:, half, :], pair_exps[0][scp][:, half, :],
                                pair_reps[0], OP.mult)
                            nc.vector.tensor_tensor(
                                scl1[:, half, :], pair_exps[1][scp][:, half, :],
                                pair_reps[1], OP.mult)
                        pr = outp.tile([P, 2, 512], BF16, name=tagpfx + "pr",
                                       tag="scl")
                        nc.vector.tensor_tensor(pr, scl0, scl1, OP.add)
                        sview = swacc[:, 2 * scp: 2 * scp + 2, :]
                        if g == 0:
                            nc.vector.tensor_copy(sview, pr)
                        else:
                            nc.vector.tensor_tensor(sview, sview, pr, OP.add)
                if g < npairs - 1:
                    swacc_work()
                else:
                    deferred.append(swacc_work)
            return aoT, deferred

        def out_proj(aoT, wo_dram, bo, resid_f32, tagpfx, res_tag, fillers,
                     stats_cb=None):
            wch = [stream_w(wproj, wo_dram, k, 0, D, tagpfx + "wo") for k in range(NC)]
            xnew = big.tile([P, NC, LQ], F32, name=tagpfx + "xres", tag=res_tag)
            for m in range(NC):
                acc = ps.tile([P, 512], F32, name=tagpfx + "ops", tag="proj")
                for k in range(NC):
                    nc.tensor.matmul(acc, wch[k][:, 128 * m: 128 * m + 128],
                                     aoT[:, k, :], start=(k == 0), stop=(k == NC - 1))
                nc.vector.scalar_tensor_tensor(
                    xnew[:, m, :], acc, bo[:, m: m + 1], resid_f32[:, m, :],
                    OP.add, OP.add)
                if stats_cb is not None and m >= 1:
                    stats_cb(xnew[:, m - 1, :], m - 1)
                if fillers:
                    fillers.popleft()()
            if stats_cb is not None:
                stats_cb(xnew[:, NC - 1, :], NC - 1)
            return xnew

        def dump_swacc(swacc, dram):
            # swacc accumulates in bf16; outputs are f32, so cast per chunk
            view = dram.rearrange("(o p) n -> p o n", p=P)
            for o in range(NSC):
                stg = outp.tile([P, 512], F32, name="swstg", tag="lnu", bufs=2)
                nc.vector.tensor_copy(stg, swacc[:, o, :])
                nc.sync.dma_start(out=view[:, o, :], in_=stg)

        def dbg_dump(tile3d, dram, n=NC):
            """Dump [P, n, 512] tile (any dtype) to a [n*P, 512] f32 dram."""
            view = dram.rearrange("(o p) n -> p o n", p=P)
            for o in range(n):
                stg = outp.tile([P, 512], F32, name="dbgstg", tag="lnu", bufs=2)
                nc.vector.tensor_copy(stg, tile3d[:, o, :])
                nc.sync.dma_start(out=view[:, o, :], in_=stg)

        # ================= pipeline =================
        x16 = big.tile([P, NC, L], BF16, name="x16", tag="bigA")
        for j in range(2):
            for o in range(NC):
                nc.sync.dma_start(
                    out=x16[:, o, 512 * j: 512 * j + 512],
                    in_=t["xT16"][P * o: P * o + P, 512 * j: 512 * j + 512])
        # cross K/V projections depend only on xa: stage their inputs now so
        # cross-K matmuls can fill the PE through LN1's serial finish chains.
        # (cross-V tiles/weights are staged after self-attention -- they
        # recycle self-attention buffers and would deadlock the tag cycle.)
        xa_b = big.tile([P, NC, L], BF16, name="xa_b", tag="lnfull2")
        k2T = big.tile([P, NC, L], BF16, name="cakT", tag="bigA")
        for o in range(NC):
            nc.sync.dma_start(out=xa_b[:, o, :],
                              in_=t["xaT16"][P * o: P * o + P, :])
        wk2 = [stream_w(wkv2, t["w_ca"], k, D, 2 * D, "cawk") for k in range(NC)]

        def k2_iter(m, j):
            def f():
                k_proj_iter(wk2, xa_b, bqk_ca, k2T, m, j)
            return f

        # (No LN1 fillers: a k2T act-write would queue behind x16's buffer
        # release and jam the whole Act queue until LN1 finishes.)
        xln_b, xlnq_f = layer_norm_full(x16, 0, "ln1", tag="lnfull",
                                        f32_tag="resB", fillers=deque())
        xlnq_b = xln_b[:, :, 0:LQ]
        if DBG == "ln":
            dbg_dump(xlnq_f, t["xoutT"])
            dbg_dump(xln_b[:, :, 512:1024], t["selfwT"])
            return

        qT = q_proj(xlnq_b, t["w_sa"], bqk_sa, "sa")
        kT = big.tile([P, NC, L], BF16, name="sakT", tag="resA")
        vnat = big.tile([P, NSC, H * VST], BF16, name="savnat", tag="vnat")
        v_ones(vnat, "sa")
        kv_proj(xln_b, t["w_sa"], bqk_sa, bv_reps["sa"], kT, vnat, "sa")
        if DBG == "qkv":
            dbg_dump(qT, t["xoutT"])
            dbg_dump(kT[:, :, 0:512], t["selfwT"])
            dbg_dump(vnat[:, :, 0:512], t["crosswT"])  # raw 65-stride layout
            return

        # cross-K spread over the self-attn pairs, one chunk per point
        # (chunk m is first consumed by cross pair m, far downstream).
        sa_points = [[k2_iter(pt // 2, pt % 2)] for pt in range(16)]

        swacc = big.tile([P, NSC, LQ], BF16, name="swacc", tag="swacc")
        aoT, sa_deferred = attention(qT, kT, vnat, swacc, "sa", sa_points)

        # ---- cross-V staging: wv2 recycles wk2's buffers and v2nat recycles
        # sa-vnat's, so these must be emitted after their last sa readers
        v2nat = big.tile([P, NSC, H * VST], BF16, name="cavnat", tag="vnat")
        v_ones(v2nat, "ca")
        wv2 = [stream_w(wkv2, t["w_ca"], k, 2 * D, 3 * D, "cawv") for k in range(NC)]

        def v2_iter(m, h0, nh):
            def f():
                v_proj_iter(wv2, xa_b, bv_reps["ca"], v2nat, m, h0, nh)
            return f

        # out-proj/LN2/Q2 window: cross-V heads 0..7 (cross pairs 0-3)
        op_fillers = deque()
        for m in range(NSC):
            op_fillers.append(v2_iter(m, 0, 4))
            op_fillers.append(v2_iter(m, 4, 4))
        # cross-attn early pairs produce cross-V for late pairs: heads 8..11
        # (consumed from pair 4, ready by point 7) and 12..15 (consumed from
        # pair 6, ready by point 11).
        ca_points = [[] for _ in range(16)]
        for m in range(NSC):
            ca_points[m].append(v2_iter(m, 8, 4))
            ca_points[4 + m].append(v2_iter(m, 12, 4))
        if DBG == "attn":
            for work in sa_deferred:
                work()
            dump_swacc(swacc, t["selfwT"])
            dbg_dump(aoT, t["crosswT"])
            return
        # ln2 output shares the aoT-tag buffer chain (sa-aoT is dead by then);
        # it must NOT share with xa_b, which cross-V fillers read much later.
        ln2_stats, ln2_finish = ln_make_split(1, "ln2", "aoT",
                                              fillers=op_fillers)
        x1 = out_proj(aoT, t["wo_sa"], bo_sa_sb, xlnq_f, "sa", "resA", op_fillers,
                      stats_cb=ln2_stats)
        x2ln_b = ln2_finish(x1)
        q2T = q_proj(x2ln_b, t["w_ca"], bqk_ca, "ca", fillers=op_fillers)
        while op_fillers:
            op_fillers.popleft()()
        # last self pair's prob-mean work lands here, overlapping Q2-proj PE
        for work in sa_deferred:
            work()
        dump_swacc(swacc, t["selfwT"])
        cwacc = big.tile([P, NSC, LQ], BF16, name="cwacc", tag="swacc")
        ao2T, ca_deferred = attention(q2T, k2T, v2nat, cwacc, "ca", ca_points)
        ln3_stats, ln3_finish = ln_make_split(2, "ln3", "aoT", warm=True)
        x2 = out_proj(ao2T, t["wo_ca"], bo_ca_sb, x1, "ca", "resB", deque(),
                      stats_cb=ln3_stats)
        x3ln_b = ln3_finish(x2)
        # last cross pair's prob-mean work overlaps FFN-f1 PE
        for work in ca_deferred:
            work()
        dump_swacc(cwacc, t["crosswT"])
        # FFN in two half-DFF waves (halves the h1 SBUF footprint); wave 0's
        # down-proj partials accumulate in SBUF f32, wave 1 adds bias+residual.
        h1 = big.tile([P, NF // 2, LQ], BF16, name="h1", tag="bigA")
        part = big.tile([P, NC, LQ], F32, name="ffnpart", tag="resA")
        for wave in range(2):
            for mg in range(2):
                cg = 2 * wave + mg
                wch = [stream_w(wproj, t["w1"], k, 1024 * cg, 1024 * cg + 1024, "w1")
                       for k in range(NC)]
                for ml in range(8):
                    m = 8 * cg + ml
                    acc = ps.tile([P, 512], F32, name="f1ps", tag="proj")
                    for k in range(NC):
                        nc.tensor.matmul(acc, wch[k][:, 128 * ml: 128 * ml + 128],
                                         x3ln_b[:, k, :], start=(k == 0),
                                         stop=(k == NC - 1))
                    nc.scalar.activation(h1[:, 8 * mg + ml, :], acc, AF.Gelu,
                                         bias=b1_sb[:, m: m + 1])
            for m in range(NC):
                acc = ps.tile([P, 512], F32, name="f2ps", tag="proj")
                for gq in range(2):
                    blk = wkv2.tile([P, 8, 128], BF16, name="w2blk", tag="wp")
                    nc.sync.dma_start(
                        out=blk,
                        in_=t["w2"][2048 * wave + 1024 * gq:
                                    2048 * wave + 1024 * gq + 1024,
                                    128 * m: 128 * m + 128].rearrange(
                            "(kk p) n -> p kk n", p=P))
                    for kk in range(8):
                        k = 8 * gq + kk
                        nc.tensor.matmul(acc, blk[:, kk, :], h1[:, k, :],
                                         start=(k == 0), stop=(k == 15))
                if wave == 0:
                    nc.vector.tensor_copy(part[:, m, :], acc)
                else:
                    xo = outp.tile([P, 512], F32, name="xo", tag="lnu", bufs=2)
                    nc.vector.scalar_tensor_tensor(
                        xo, acc, b2_sb[:, m: m + 1], part[:, m, :],
                        OP.add, OP.add)
                    xo2 = outp.tile([P, 512], F32, name="xo2", tag="lnv", bufs=2)
                    nc.vector.tensor_tensor(xo2, xo, x2[:, m, :], OP.add)
                    nc.sync.dma_start(
                        out=t["xoutT"].rearrange("(o p) n -> p o n", p=P)[:, m, :],
                        in_=xo2)


_NC_CACHE = {}


def _get_nc():
    if "nc" not in _NC_CACHE:
        _NC_CACHE["nc"] = _build()
    return _NC_CACHE["nc"]


def kernel(**inputs):
    inp = {k: np.asarray(v, dtype=np.float32) for k, v in inputs.items()}

    def bt(a):  # transpose + bf16
        return np.ascontiguousarray(a.T).astype(ml_dtypes.bfloat16)

    shared = {
        "w_sa": bt(inp["sa_in_w"]), "b_sa": inp["sa_in_b"],
        "wo_sa": bt(16.0 * inp["sa_out_w"]), "bo_sa": inp["sa_out_b"],
        "w_ca": bt(inp["ca_in_w"]), "b_ca": inp["ca_in_b"],
        "wo_ca": bt(16.0 * inp["ca_out_w"]), "bo_ca": inp["ca_out_b"],
        "w1": bt(inp["ff_w1"]), "b1": inp["ff_b1"],
        "w2": bt(inp["ff_w2"]), "b2": inp["ff_b2"],
        "ln_w": np.ascontiguousarray(
            np.stack([inp["ln1_w"], inp["ln2_w"], inp["ln3_w"]])),
        "ln_b": np.ascontiguousarray(
            np.stack([inp["ln1_b"], inp["ln2_b"], inp["ln3_b"]])),
    }
    perms = []
    in_maps = []
    for c in range(8):
        b, r = c // 2, c % 2
        perm = np.r_[512 * r: 512 * r + 512, 512 * (1 - r): 512 * (1 - r) + 512]
        perms.append(perm)
        in_maps.append({
            "xT16": np.ascontiguousarray(
                inp["x"][b][perm].T).astype(ml_dtypes.bfloat16),
            "xaT16": np.ascontiguousarray(inp["xa"][b].T).astype(ml_dtypes.bfloat16),
            **shared,
        })

    res = run_bass_kernel_spmd(_get_nc(), in_maps, core_ids=list(range(8)))

    x = np.empty((B, L, D), np.float32)
    self_w = np.empty((B, L, L), np.float32)
    cross_w = np.empty((B, L, L), np.float32)
    for c in range(8):
        b, r = c // 2, c % 2
        rows = slice(512 * r, 512 * r + 512)
        x[b, rows] = res.results[c]["xoutT"].T
        # b (int) + perm (array) are both advanced indices separated by a
        # slice, so numpy puts the perm dim first: target shape (1024, 512)
        # with semantics self_w[b, l, perm[j]] = selfwT[j, l].
        self_w[b, rows.start: rows.stop, perms[c]] = res.results[c]["selfwT"]
        cross_w[b, rows] = res.results[c]["crosswT"].T
    return (x, self_w, cross_w)



# revision 12
# speedup vs baseline: 2.6147x; 2.6147x over previous
"""Trainium2 Bass kernel for nn_DecoderLayer (self-attn + cross-attn + FFN).

Sharding: 8 cores, no collectives. Core c handles batch b=c//2, query-row
half r=c%2 (512 of 1024 rows). All per-core differences flow through input
data (host slices/transposes/permutes), so one SPMD NEFF serves all cores.

On-device layout is feature-major ("transposed"): activations live as
[channels(partitions), tokens(free)]. Weights are host-pre-transposed to
[in_ch, out_ch] and cast to bf16. Matmul operands are bf16 (fp32 PSUM
accumulation); the residual stream stays fp32.

x is passed column-PERMUTED so this core's 512 query tokens are always
columns 0:512 -- attention is permutation-equivariant over key positions,
and the host un-permutes the returned attention-weight rows.

Scheduling devices (PE clock gate wants a dense matmul stream; DVE is the
binding engine during attention):
- V bias is folded into the out-proj bias on the HOST (bo' = bo + Wo@bv,
  exact because softmax rows sum to 1), so V write-back is a plain copy.
- V is stored per head-PAIR as [ch_e(64) den_e den_o ch_o(64)] with 16.0
  denominator columns, so each attn@V also accumulates that head's softmax
  denominator (x16) into PSUM; odd heads write PSUM rows 63:128 so their
  normalized output lands on aoT partitions 64:128 directly (no DMA shift).
- swacc (head-mean probs) accumulates via gpsimd SWDGE DMA-accumulate,
  keeping roughly half the accumulation passes off the DVE.
- LN1 writes its output in place over x16; the freed buffer holds the
  cross-K projection, whose chunks fill the PE during LN1's serial finish.
- LN finish chains split chunks across DVE and gpsimd; LN3's finish
  interleaves the first FFN-w1 accumulation chunks (PSUM is free there).
- Scores accumulate into 2-bank PSUM tiles so exp runs 1024 wide.
"""

from collections import deque

import ml_dtypes
import numpy as np

import concourse.bacc as bacc
import concourse.mybir as mybir
import concourse.tile as tile
from concourse.bass_utils import run_bass_kernel_spmd

F32 = mybir.dt.float32
BF16 = mybir.dt.bfloat16
AF = mybir.ActivationFunctionType
OP = mybir.AluOpType

P = 128
D = 1024
DFF = 4096
H = 16
B = 4
L = 1024          # full sequence (keys/values)
LQ = 512          # per-core query tokens
NC = D // P       # 8 channel chunks
NF = DFF // P     # 32 ff chunks
NSC = L // P      # 8 key-position chunks
VST = 65          # per-head V stride (64 channels + 1 denominator column)
PST = 2 * VST     # per head-pair V stride: [ch_e(64) den_e den_o ch_o(64)]
EPS = 1e-5


def _build():
    nc = bacc.Bacc("TRN2", target_bir_lowering=False)

    xT16 = nc.dram_tensor("xT16", [D, L], BF16, kind="ExternalInput")    # permuted x[b].T bf16
    xaT16 = nc.dram_tensor("xaT16", [D, L], BF16, kind="ExternalInput")  # xa[b].T bf16
    w_sa = nc.dram_tensor("w_sa", [D, 3 * D], BF16, kind="ExternalInput")
    bqk_sa = nc.dram_tensor("bqk_sa", [P, 16], F32, kind="ExternalInput")
    wo_sa = nc.dram_tensor("wo_sa", [D, D], BF16, kind="ExternalInput")
    bo_sa = nc.dram_tensor("bo_sa", [P, NC], F32, kind="ExternalInput")
    w_ca = nc.dram_tensor("w_ca", [D, 3 * D], BF16, kind="ExternalInput")
    bqk_ca = nc.dram_tensor("bqk_ca", [P, 16], F32, kind="ExternalInput")
    wo_ca = nc.dram_tensor("wo_ca", [D, D], BF16, kind="ExternalInput")
    bo_ca = nc.dram_tensor("bo_ca", [P, NC], F32, kind="ExternalInput")
    w1 = nc.dram_tensor("w1", [D, DFF], BF16, kind="ExternalInput")
    b1 = nc.dram_tensor("b1", [P, NF], F32, kind="ExternalInput")
    w2 = nc.dram_tensor("w2", [DFF, D], BF16, kind="ExternalInput")
    b2 = nc.dram_tensor("b2", [P, NC], F32, kind="ExternalInput")
    ln_w = nc.dram_tensor("ln_w", [P, 3, NC], F32, kind="ExternalInput")
    ln_b = nc.dram_tensor("ln_b", [P, 3, NC], F32, kind="ExternalInput")

    xoutT = nc.dram_tensor("xoutT", [D, LQ], F32, kind="ExternalOutput")
    selfwT = nc.dram_tensor("selfwT", [L, LQ], F32, kind="ExternalOutput")
    crosswT = nc.dram_tensor("crosswT", [L, LQ], F32, kind="ExternalOutput")

    with tile.TileContext(nc) as tc:
        _emit(nc, tc, locals())
    nc.compile()
    return nc


def _emit(nc, tc, t):
    import contextlib
    import os
    DBG = os.environ.get("KBDBG", "")
    ctx = contextlib.ExitStack()
    with ctx:
        const = ctx.enter_context(tc.tile_pool(name="const", bufs=1))
        big = ctx.enter_context(tc.tile_pool(name="big", bufs=1))
        wproj = ctx.enter_context(tc.tile_pool(name="wproj", bufs=8))
        wkv2 = ctx.enter_context(tc.tile_pool(name="wkv2", bufs=8))
        sm = ctx.enter_context(tc.tile_pool(name="sm", bufs=3))     # [.,512] rows
        rep = ctx.enter_context(tc.tile_pool(name="rep", bufs=2))   # broadcast tiles
        expp = ctx.enter_context(tc.tile_pool(name="expp", bufs=9))  # [P,2,512] prob tiles
        outp = ctx.enter_context(tc.tile_pool(name="outp", bufs=3))  # transient tiles
        ps = ctx.enter_context(tc.tile_pool(name="ps", bufs=2, space="PSUM"))
        ps_s = ctx.enter_context(tc.tile_pool(name="ps_s", bufs=2, space="PSUM"))
        ps_av = ctx.enter_context(tc.tile_pool(name="ps_av", bufs=2, space="PSUM"))

        # ---- input DMAs: batched (one HWDGE config each), x16 kv-half
        # first so LN1 stats can start asap ----
        xt_v = t["xT16"].rearrange("(o p) n -> p o n", p=P)
        x16 = big.tile([P, NC, L], BF16, name="x16", tag="bigA")
        nc.sync.dma_start(out=x16[:, :, 512:1024], in_=xt_v[:, :, 512:1024])
        lnw_sb = const.tile([P, 3, NC], F32, name="lnw_sb")
        nc.sync.dma_start(out=lnw_sb, in_=t["ln_w"][:, :, :])
        lnb_sb = const.tile([P, 3, NC], F32, name="lnb_sb")
        nc.sync.dma_start(out=lnb_sb, in_=t["ln_b"][:, :, :])
        nc.sync.dma_start(out=x16[:, :, 0:512], in_=xt_v[:, :, 0:512])
        bqk_sa = const.tile([P, 16], F32, name="bqk_sa")
        nc.sync.dma_start(out=bqk_sa, in_=t["bqk_sa"][:, :])
        # cross K/V projections depend only on xa: stage their inputs now so
        # cross-K matmuls can fill the PE through LN1's serial finish chains.
        xa_b = big.tile([P, NC, L], BF16, name="xa_b", tag="xab")
        nc.sync.dma_start(out=xa_b[:, :, :],
                          in_=t["xaT16"].rearrange("(o p) n -> p o n", p=P))
        bqk_ca = const.tile([P, 16], F32, name="bqk_ca")
        nc.sync.dma_start(out=bqk_ca, in_=t["bqk_ca"][:, :])

        def stream_w(pool, dram, k, lo, hi, name):
            w_t = pool.tile([P, hi - lo], BF16, name=name, tag="wp")
            nc.sync.dma_start(out=w_t, in_=dram[P * k: P * k + P, lo:hi])
            return w_t

        wk2 = [stream_w(wkv2, t["w_ca"], k, D, 2 * D, "cawk") for k in range(NC)]
        bo_sa_sb = const.tile([P, NC], F32, name="bo_sa_sb")
        nc.sync.dma_start(out=bo_sa_sb, in_=t["bo_sa"][:, :])
        bo_ca_sb = const.tile([P, NC], F32, name="bo_ca_sb")
        nc.sync.dma_start(out=bo_ca_sb, in_=t["bo_ca"][:, :])
        b1_sb = const.tile([P, NF], F32, name="b1_sb")
        nc.sync.dma_start(out=b1_sb, in_=t["b1"][:, :])
        b2_sb = const.tile([P, NC], F32, name="b2_sb")
        nc.sync.dma_start(out=b2_sb, in_=t["b2"][:, :])
        ones_sb = const.tile([P, 1], BF16, name="ones_sb")
        nc.vector.memset(ones_sb, 1.0)
        eps_col = const.tile([1, 1], F32, name="eps_col")
        nc.vector.memset(eps_col, EPS)

        # cross-K output: own buffer (freed by LN1 running in place) so the
        # k2 fillers can run during LN1.
        k2T = big.tile([P, NC, L], BF16, name="cakT", tag="k2T")

        # ---------------- layer norm helpers ----------------
        def ln_stats_tile(name):
            """One PSUM bank holding sum (row 0) and sum-sq (row 32)."""
            return ps_av.tile([33, 512], F32, name=name, tag="av")

        def ln_stats_chunk(st, x_chunk, o, name):
            xb = outp.tile([P, 512], BF16, name=name + "_xb", tag="lnt", bufs=2)
            nc.vector.tensor_copy(xb, x_chunk)
            sq = outp.tile([P, 512], BF16, name=name + "_sq", tag="lnt", bufs=2)
            nc.scalar.activation(sq, x_chunk, AF.Square)
            nc.tensor.matmul(st[0:1, :], ones_sb, xb,
                             start=(o == 0), stop=(o == NC - 1),
                             skip_group_check=True)
            nc.tensor.matmul(st[32:33, :], ones_sb, sq,
                             start=(o == 0), stop=(o == NC - 1),
                             skip_group_check=True)

        def ln_reduce(st, name):
            """stats psum -> (mean_rep, rsq_rep) f32 broadcast tiles."""
            mean = sm.tile([1, 512], F32, name=name + "_mean", tag="row")
            nc.vector.tensor_scalar_mul(mean, st[0:1, :], 1.0 / D)
            mean_rep = rep.tile([P, 512], F32, name=name + "_mrep", tag="rep")
            nc.gpsimd.partition_broadcast(mean_rep, mean)
            m2 = sm.tile([1, 512], F32, name=name + "_m2", tag="row")
            nc.vector.tensor_tensor(m2, mean, mean, OP.mult)
            var = sm.tile([1, 512], F32, name=name + "_var", tag="row")
            nc.vector.scalar_tensor_tensor(var, st[32:33, :], 1.0 / D, m2,
                                           OP.mult, OP.subtract)
            rsq = sm.tile([1, 512], F32, name=name + "_rsq", tag="row")
            nc.scalar.activation(rsq, var, AF.Abs_reciprocal_sqrt, bias=eps_col)
            rsq_rep = rep.tile([P, 512], F32, name=name + "_rrep", tag="rep")
            nc.gpsimd.partition_broadcast(rsq_rep, rsq)
            return mean_rep, rsq_rep

        def ln_finish_chunk(x_chunk, mean_rep, rsq_rep, ln_idx, o, out_b_chunk,
                            name, out_f_chunk=None, eng=None):
            eng = eng or nc.vector
            u = outp.tile([P, 512], F32, name=name + "_u", tag="lnu", bufs=2)
            eng.tensor_tensor(u, x_chunk, mean_rep, OP.subtract)
            v = outp.tile([P, 512], F32, name=name + "_v", tag="lnv", bufs=2)
            eng.tensor_tensor(v, u, rsq_rep, OP.mult)
            # ln weight folds into the Act per-partition scale
            if out_f_chunk is not None:
                nc.scalar.activation(out_f_chunk, v, AF.Identity,
                                     bias=lnb_sb[:, ln_idx, o: o + 1],
                                     scale=lnw_sb[:, ln_idx, o: o + 1])
                eng.tensor_copy(out_b_chunk, out_f_chunk)
            else:
                nc.scalar.activation(out_b_chunk, v, AF.Identity,
                                     bias=lnb_sb[:, ln_idx, o: o + 1],
                                     scale=lnw_sb[:, ln_idx, o: o + 1])

        def chunk_eng(o):
            # chunks 2 and 5 run on gpsimd, the rest on DVE
            return nc.gpsimd if o in (2, 5) else nc.vector

        def layer_norm_full(x16, ln_idx, name, f32_tag, fillers):
            """In-place LN over [P, NC, L]: x16 is overwritten chunk by chunk.
            Also writes the f32 query half to a fresh tile (residual base).
            kv half (j=1) runs first; its serial finish chain overlaps the
            query half's stats matmuls. Fillers run one per finish chunk."""
            out_f = big.tile([P, NC, LQ], F32, name=name + "_lnf", tag=f32_tag)

            def stats(j):
                sl = slice(512 * j, 512 * j + 512)
                st = ps_s.tile([33, 512], F32, name=name + "_st", tag="sc")
                for o in range(NC):
                    sq = outp.tile([P, 512], BF16, name=name + "_sq", tag="lnt", bufs=2)
                    nc.scalar.activation(sq, x16[:, o, sl], AF.Square)
                    nc.tensor.matmul(st[0:1, :], ones_sb, x16[:, o, sl],
                                     start=(o == 0), stop=(o == NC - 1),
                                     skip_group_check=True)
                    nc.tensor.matmul(st[32:33, :], ones_sb, sq,
                                     start=(o == 0), stop=(o == NC - 1),
                                     skip_group_check=True)
                return st

            st1 = stats(1)
            mean_rep, rsq_rep = ln_reduce(st1, name)
            st0 = stats(0)  # query-half stats matmuls fill the j=1 finish
            for o in range(NC):
                ln_finish_chunk(x16[:, o, 512:1024], mean_rep, rsq_rep, ln_idx,
                                o, x16[:, o, 512:1024], name, eng=chunk_eng(o))
                if fillers:
                    fillers.popleft()()
            mean_rep, rsq_rep = ln_reduce(st0, name)
            for o in range(NC):
                ln_finish_chunk(x16[:, o, 0:512], mean_rep, rsq_rep, ln_idx,
                                o, x16[:, o, 0:512], name,
                                out_f_chunk=out_f[:, o, :], eng=chunk_eng(o))
                if fillers:
                    fillers.popleft()()
            return x16, out_f

        def ln_make_split(ln_idx, name, tag, fillers=None):
            """Split LN over [P, NC, 512]: returns (stats_chunk, finish).
            Fillers are popped one per finish chunk and called as
            f(out_b, o) -- closures may ignore the args."""
            st_box = {}

            def stats_chunk(x_chunk, o):
                if "st" not in st_box:
                    st_box["st"] = ln_stats_tile(name + "_st")
                ln_stats_chunk(st_box["st"], x_chunk, o, name)

            def finish(x_sb):
                out_b = big.tile([P, NC, 512], BF16, name=name + "_ln", tag=tag)
                mean_rep, rsq_rep = ln_reduce(st_box["st"], name)
                for o in range(NC):
                    ln_finish_chunk(x_sb[:, o, :], mean_rep, rsq_rep, ln_idx, o,
                                    out_b[:, o, :], name, eng=chunk_eng(o))
                    if fillers:
                        fillers.popleft()(out_b, o)
                return out_b

            return stats_chunk, finish

        # ---------------- projections ----------------
        def q_proj(xq_b, w_dram, bqk, tagpfx, fillers=None):
            qT = big.tile([P, NC, LQ], BF16, name=tagpfx + "qT", tag="qT")
            wch = [stream_w(wproj, w_dram, k, 0, D, tagpfx + "wq") for k in range(NC)]
            for m in range(NC):
                acc = ps.tile([P, 512], F32, name=tagpfx + "qps", tag="proj")
                for k in range(NC):
                    nc.tensor.matmul(acc, wch[k][:, 128 * m: 128 * m + 128],
                                     xq_b[:, k, :], start=(k == 0), stop=(k == NC - 1))
                nc.scalar.activation(qT[:, m, :], acc, AF.Identity, bias=bqk[:, m: m + 1])
                if fillers and m % 2 == 1:
                    fillers.popleft()(None, m)
            return qT

        def k_proj_iter(wch, xkv_b, bqk, kT, m, j):
            acc = ps.tile([P, 512], F32, name="kps", tag="proj")
            for k in range(NC):
                nc.tensor.matmul(
                    acc, wch[k][:, 128 * m: 128 * m + 128],
                    xkv_b[:, k, 512 * j: 512 * j + 512],
                    start=(k == 0), stop=(k == NC - 1))
            nc.scalar.activation(kT[:, m, 512 * j: 512 * j + 512], acc,
                                 AF.Identity, bias=bqk[:, 8 + m: 9 + m])

        def v_proj_iter(wch, xkv_b, vnat, m, h0, nh):
            """V proj for heads h0..h0+nh, key-chunk m. Per-head layout
            [ch(64) den] at stride VST; bias is folded into out-proj."""
            acc = ps.tile([P, 64 * nh], F32, name="vps", tag="proj")
            for k in range(NC):
                nc.tensor.matmul(
                    acc, xkv_b[:, k, 128 * m: 128 * m + 128],
                    wch[k][:, 64 * h0: 64 * h0 + 64 * nh],
                    start=(k == 0), stop=(k == NC - 1))
            base = VST * h0
            vv = vnat[:, m, base: base + VST * nh].rearrange(
                "p (h c) -> p h c", c=VST)[:, :, 0:64]
            nc.vector.tensor_copy(vv, acc.rearrange("p (h c) -> p h c", c=64))

        def v_ones(vnat, name):
            # 16.0 in every head's denominator column: folds the 1/H
            # head-mean into the reciprocal (out-proj weights are
            # host-scaled by 16).
            view = vnat.rearrange("p s (h c) -> p s h c", c=VST)[:, :, :, 64]
            nc.vector.memset(view, 16.0)

        def kv_proj(xkv_b, w_dram, bqk, kT, vnat, tagpfx):
            wch = [stream_w(wproj, w_dram, k, D, 2 * D, tagpfx + "wk")
                   for k in range(NC)]
            for m in range(NC):
                for j in range(2):
                    k_proj_iter(wch, xkv_b, bqk, kT, m, j)
            wch = [stream_w(wproj, w_dram, k, 2 * D, 3 * D, tagpfx + "wv")
                   for k in range(NC)]
            for m in range(NSC):
                for h0 in range(0, H, 8):
                    v_proj_iter(wch, xkv_b, vnat, m, h0, 8)

        # ---------------- attention ----------------
        def attention(qT, kT, vnat, swacc, tagpfx, points):
            """Returns aoT [P, NC, LQ] bf16 (normalized attn out, transposed).
            Accumulates head-mean probs into swacc [P, NSC, LQ] bf16 via
            DVE + gpsimd DMA-accumulate. points: 16 lists of filler closures
            run at point 2g (after pair g's scores) and 2g+1 (mid-AV)."""
            aoT = big.tile([P, NC, LQ], BF16, name=tagpfx + "aoT", tag="aoT")
            deferred = []
            npairs = H // 2

            def pop_fillers(pt):
                for f in points[pt]:
                    f()

            for g in range(npairs):
                pair_exps = []   # per head: list of 4 [P,2,512] bf16 tiles
                # scores + exp for both heads
                for hh in range(2):
                    h = 2 * g + hh
                    base = 64 * hh
                    exps = []
                    for scp in range(4):
                        pss = ps_s.tile([P, 2, 512], F32, name=tagpfx + "pss", tag="sc")
                        for half in range(2):
                            sc = 2 * scp + half
                            nc.tensor.matmul(
                                pss[:, half, :],
                                kT[base: base + 64, g, 128 * sc: 128 * sc + 128],
                                qT[base: base + 64, g, :],
                                start=True, stop=True, skip_group_check=True)
                        e = expp.tile([P, 2, 512], BF16, name=tagpfx + "exp", tag="exp")
                        nc.scalar.activation(e, pss, AF.Exp, scale=0.125)
                        exps.append(e)
                    pair_exps.append(exps)
                # independent PE work bridging the exp (Act) latency
                pop_fillers(2 * g)
                # attn@V with fused denominator (row 64), both heads; each
                # head's reciprocal chain starts right after its pav so the
                # broadcast overlaps the other head's AV matmuls.
                pair_pav = []
                pair_reps = []
                for hh in range(2):
                    h = 2 * g + hh
                    pav = ps_av.tile([65, 512], F32, name=tagpfx + "pav", tag="av")
                    for sc in range(NSC):
                        nc.tensor.matmul(
                            pav, vnat[:, sc, VST * h: VST * h + VST],
                            pair_exps[hh][sc // 2][:, sc % 2, :],
                            start=(sc == 0), stop=(sc == NSC - 1),
                            skip_group_check=True)
                    pair_pav.append(pav)
                    # custom-DVE recip mis-reads PSUM inputs: stage via SBUF
                    den = sm.tile([1, 512], F32, name=tagpfx + "den", tag="row")
                    nc.scalar.activation(den, pav[64:65, :], AF.Identity)
                    rec = sm.tile([1, 512], F32, name=tagpfx + "rec", tag="row")
                    nc.vector.reciprocal_approx_fast(rec, den)
                    rec16 = sm.tile([1, 512], BF16, name=tagpfx + "rec16",
                                    tag="row16", bufs=2)
                    nc.gpsimd.tensor_copy(rec16, rec)
                    rec_rep = rep.tile([P, 512], BF16, name=tagpfx + "rrep",
                                       tag="rep16", bufs=2)
                    nc.gpsimd.partition_broadcast(rec_rep, rec16)
                    pair_reps.append(rec_rep)
                    if hh == 0:
                        pop_fillers(2 * g + 1)

                def swacc_work(g=g, pair_exps=pair_exps, pair_reps=pair_reps):
                    # head-mean prob accumulation: DVE scales + pair-adds,
                    # gpsimd SWDGE DMA-accumulate into swacc.
                    r0 = pair_reps[0].unsqueeze(1).broadcast_to([P, 2, 512])
                    r1 = pair_reps[1].unsqueeze(1).broadcast_to([P, 2, 512])
                    for scp in range(4):
                        s0 = outp.tile([P, 2, 512], BF16, name=tagpfx + "s0",
                                       tag="scl")
                        nc.vector.tensor_tensor(s0, pair_exps[0][scp], r0, OP.mult)
                        s1 = outp.tile([P, 2, 512], BF16, name=tagpfx + "s1",
                                       tag="scl")
                        nc.vector.tensor_tensor(s1, pair_exps[1][scp], r1, OP.mult)
                        sview = swacc[:, 2 * scp: 2 * scp + 2, :]
                        if g == 0:
                            nc.vector.tensor_tensor(sview, s0, s1, OP.add)
                        else:
                            pr = outp.tile([P, 2, 512], BF16, name=tagpfx + "pr",
                                           tag="prD", bufs=2)
                            nc.vector.tensor_tensor(pr, s0, s1, OP.add)
                            eng = nc.gpsimd if scp >= 3 else nc.vector
                            eng.tensor_tensor(sview, sview, pr, OP.add)
                # prob-mean first: its DVE ops release the exp tiles, which
                # gate the next pair's scores via the expp WAR chain.
                if g < npairs - 1:
                    swacc_work()
                else:
                    deferred.append(swacc_work)
                # normalized attention out: head0 direct, head1 via SWDGE
                # partition-shift (PSUM rows 0:64 -> aoT partitions 64:128)
                nc.vector.tensor_tensor(
                    aoT[0:64, g, :], pair_pav[0][0:64, :],
                    pair_reps[0][0:64, :], OP.mult)
                tmp = outp.tile([64, 512], BF16, name=tagpfx + "aotmp",
                                tag="aotmp", bufs=2)
                nc.vector.tensor_tensor(
                    tmp, pair_pav[1][0:64, :], pair_reps[1][0:64, :], OP.mult)
                nc.gpsimd.dma_start(out=aoT[64:128, g, :], in_=tmp)
            return aoT, deferred

        def out_proj(aoT, wch, bo, resid_f32, tagpfx, res_tag, fillers,
                     stats_cb=None):
            xnew = big.tile([P, NC, LQ], F32, name=tagpfx + "xres", tag=res_tag)
            for m in range(NC):
                acc = ps.tile([P, 512], F32, name=tagpfx + "ops", tag="proj")
                for k in range(NC):
                    nc.tensor.matmul(acc, wch[k][:, 128 * m: 128 * m + 128],
                                     aoT[:, k, :], start=(k == 0), stop=(k == NC - 1))
                nc.vector.scalar_tensor_tensor(
                    xnew[:, m, :], acc, bo[:, m: m + 1], resid_f32[:, m, :],
                    OP.add, OP.add)
                if stats_cb is not None and m >= 1:
                    stats_cb(xnew[:, m - 1, :], m - 1)
                if fillers:
                    fillers.popleft()(None, m)
            if stats_cb is not None:
                stats_cb(xnew[:, NC - 1, :], NC - 1)
            return xnew

        def dump_swacc(swacc, dram):
            # swacc accumulates in bf16; outputs are f32, so cast per chunk
            view = dram.rearrange("(o p) n -> p o n", p=P)
            for o in range(NSC):
                stg = outp.tile([P, 512], F32, name="swstg", tag="lnu", bufs=2)
                nc.vector.tensor_copy(stg, swacc[:, o, :])
                nc.sync.dma_start(out=view[:, o, :], in_=stg)

        def dbg_dump(tile3d, dram, n=NC):
            view = dram.rearrange("(o p) n -> p o n", p=P)
            for o in range(n):
                stg = outp.tile([P, 512], F32, name="dbgstg", tag="lnu", bufs=2)
                nc.vector.tensor_copy(stg, tile3d[:, o, :])
                nc.sync.dma_start(out=view[:, o, :], in_=stg)

        # ================= pipeline =================
        def k2_iter(m, j):
            def f(*_):
                k_proj_iter(wk2, xa_b, bqk_ca, k2T, m, j)
            return f

        # LN1 in place; cross-K chunks m=0..3 fill the finish chains
        ln1_fillers = deque(k2_iter(m, j) for m in range(4) for j in range(2))
        xln_b, xlnq_f = layer_norm_full(x16, 0, "ln1", f32_tag="resB",
                                        fillers=ln1_fillers)
        xlnq_b = xln_b[:, :, 0:LQ]
        if DBG == "ln":
            dbg_dump(xlnq_f, t["xoutT"])
            dbg_dump(xln_b[:, :, 512:1024], t["selfwT"])
            return

        qT = q_proj(xlnq_b, t["w_sa"], bqk_sa, "sa")
        kT = big.tile([P, NC, L], BF16, name="sakT", tag="resA")
        vnat = big.tile([P, NSC, H * VST], BF16, name="savnat", tag="vnat")
        v_ones(vnat, "sa")
        kv_proj(xln_b, t["w_sa"], bqk_sa, kT, vnat, "sa")
        # prefetch self out-proj weights (wproj is idle through attention)
        wo_sa_ch = [stream_w(wproj, t["wo_sa"], k, 0, D, "sawo") for k in range(NC)]
        if DBG == "qkv":
            dbg_dump(qT, t["xoutT"])
            dbg_dump(kT[:, :, 0:512], t["selfwT"])
            dbg_dump(vnat[:, :, 0:512], t["crosswT"])
            return

        # remaining cross-K chunks spread over the self-attn even points
        sa_points = [[] for _ in range(16)]
        for i, (m, j) in enumerate([(m, j) for m in range(4, 8) for j in range(2)]):
            sa_points[2 * i].append(k2_iter(m, j))

        swacc = big.tile([P, NSC, LQ], BF16, name="swacc", tag="swacc")
        aoT, sa_deferred = attention(qT, kT, vnat, swacc, "sa", sa_points)

        # ---- cross-V staging: wv2 recycles wk2's buffers and v2nat recycles
        # sa-vnat's, so these must be emitted after their last sa readers
        v2nat = big.tile([P, NSC, H * VST], BF16, name="cavnat", tag="vnat")
        v_ones(v2nat, "ca")
        wv2 = [stream_w(wkv2, t["w_ca"], k, 2 * D, 3 * D, "cawv") for k in range(NC)]

        def v2_iter(m, h0, nh):
            def f(*_):
                v_proj_iter(wv2, xa_b, v2nat, m, h0, nh)
            return f

        # out-proj/LN2/Q2 window: cross-V heads 0..7 (cross pairs 0-3)
        op_fillers = deque()
        for m in range(NSC):
            op_fillers.append(v2_iter(m, 0, 4))
            op_fillers.append(v2_iter(m, 4, 4))
        # cross-attn early pairs produce cross-V for late pairs: heads 8..11
        # (consumed from pair 4) and 12..15 (consumed from pair 6).
        ca_points = [[] for _ in range(16)]
        for m in range(NSC):
            ca_points[m].append(v2_iter(m, 8, 4))
            ca_points[4 + m].append(v2_iter(m, 12, 4))
        if DBG == "attn":
            for work in sa_deferred:
                work()
            dump_swacc(swacc, t["selfwT"])
            dbg_dump(aoT, t["crosswT"])
            return
        # ln2 output shares the aoT-tag buffer chain (sa-aoT is dead by then)
        ln2_stats, ln2_finish = ln_make_split(1, "ln2", "aoT",
                                              fillers=op_fillers)
        x1 = out_proj(aoT, wo_sa_ch, bo_sa_sb, xlnq_f, "sa", "resA", op_fillers,
                      stats_cb=ln2_stats)
        x2ln_b = ln2_finish(x1)
        q2T = q_proj(x2ln_b, t["w_ca"], bqk_ca, "ca", fillers=op_fillers)
        while op_fillers:
            op_fillers.popleft()(None, 0)
        # last self pair's prob-mean work lands here, overlapping Q2-proj PE
        for work in sa_deferred:
            work()
        # prefetch cross out-proj weights before cross attention
        wo_ca_ch = [stream_w(wproj, t["wo_ca"], k, 0, D, "cawo") for k in range(NC)]
        dump_swacc(swacc, t["selfwT"])
        cwacc = big.tile([P, NSC, LQ], BF16, name="cwacc", tag="swacc")
        ao2T, ca_deferred = attention(q2T, k2T, v2nat, cwacc, "ca", ca_points)

        # FFN w1 first-block weights (into wkv2: wk2 is dead) + fill
        # accumulators: LN3's finish interleaves the first 4 m-chunks of the
        # FFN up-projection (PSUM banks are free here).
        w1c0 = [stream_w(wkv2, t["w1"], k, 0, 1024, "w1c0") for k in range(NC)]
        ffn_box = {}

        def ffn_fill(out_b, o):
            if "a" not in ffn_box:
                ffn_box["a"] = [ps_s.tile([P, 2, 512], F32, name="f1pre", tag="sc")
                                for _ in range(2)]
            for mi in range(4):
                nc.tensor.matmul(
                    ffn_box["a"][mi // 2][:, mi % 2, :],
                    w1c0[o][:, 128 * mi: 128 * mi + 128], out_b[:, o, :],
                    start=(o == 0), stop=(o == NC - 1), skip_group_check=True)

        ln3_stats, ln3_finish = ln_make_split(2, "ln3", "aoT",
                                              fillers=deque([ffn_fill] * NC))
        x2 = out_proj(ao2T, wo_ca_ch, bo_ca_sb, x1, "ca", "resB", deque(),
                      stats_cb=ln3_stats)
        x3ln_b = ln3_finish(x2)
        # last cross pair's prob-mean work overlaps FFN-f1 PE
        for work in ca_deferred:
            work()
        dump_swacc(cwacc, t["crosswT"])
        # FFN in two half-DFF waves (halves the h1 SBUF footprint); wave 0's
        # down-proj partials accumulate in SBUF f32, wave 1 adds bias+residual.
        h1 = big.tile([P, NF // 2, LQ], BF16, name="h1", tag="bigA")
        part = big.tile([P, NC, LQ], F32, name="ffnpart", tag="resA")
        for wave in range(2):
            for mg in range(2):
                cg = 2 * wave + mg
                if cg == 0:
                    # m-chunks 0..3 were pre-accumulated during LN3's finish
                    for ml in range(4):
                        nc.scalar.activation(
                            h1[:, ml, :], ffn_box["a"][ml // 2][:, ml % 2, :],
                            AF.Gelu, bias=b1_sb[:, ml: ml + 1])
                    wch = w1c0
                    mls = range(4, 8)
                else:
                    wch = [stream_w(wproj, t["w1"], k, 1024 * cg, 1024 * cg + 1024,
                                    "w1") for k in range(NC)]
                    mls = range(8)
                for ml in mls:
                    m = 8 * cg + ml
                    acc = ps.tile([P, 512], F32, name="f1ps", tag="proj")
                    for k in range(NC):
                        nc.tensor.matmul(acc, wch[k][:, 128 * ml: 128 * ml + 128],
                                         x3ln_b[:, k, :], start=(k == 0),
                                         stop=(k == NC - 1))
                    nc.scalar.activation(h1[:, 8 * mg + ml, :], acc, AF.Gelu,
                                         bias=b1_sb[:, m: m + 1])
            for m in range(NC):
                acc = ps.tile([P, 512], F32, name="f2ps", tag="proj")
                for gq in range(2):
                    blk = wkv2.tile([P, 8, 128], BF16, name="w2blk", tag="wp")
                    nc.sync.dma_start(
                        out=blk,
                        in_=t["w2"][2048 * wave + 1024 * gq:
                                    2048 * wave + 1024 * gq + 1024,
                                    128 * m: 128 * m + 128].rearrange(
                            "(kk p) n -> p kk n", p=P))
                    for kk in range(8):
                        k = 8 * gq + kk
                        nc.tensor.matmul(acc, blk[:, kk, :], h1[:, k, :],
                                         start=(k == 0), stop=(k == 15))
                if wave == 0:
                    nc.vector.tensor_copy(part[:, m, :], acc)
                else:
                    xo = outp.tile([P, 512], F32, name="xo", tag="lnu", bufs=2)
                    nc.vector.scalar_tensor_tensor(
                        xo, acc, b2_sb[:, m: m + 1], part[:, m, :],
                        OP.add, OP.add)
                    xo2 = outp.tile([P, 512], F32, name="xo2", tag="lnv", bufs=2)
                    nc.vector.tensor_tensor(xo2, xo, x2[:, m, :], OP.add)
                    nc.scalar.dma_start(
                        out=t["xoutT"].rearrange("(o p) n -> p o n", p=P)[:, m, :],
                        in_=xo2)


_NC_CACHE = {}


def _get_nc():
    if "nc" not in _NC_CACHE:
        _NC_CACHE["nc"] = _build()
    return _NC_CACHE["nc"]


def _col_major(v, n):
    # channel c lives at [partition c % 128, col c // 128]
    return np.ascontiguousarray(np.asarray(v, np.float32).reshape(n, P).T)


def build_in_maps(inputs):
    inp = {k: np.asarray(v, dtype=np.float32) for k, v in inputs.items()}

    def bt(a):  # transpose + bf16
        return np.ascontiguousarray(a.T).astype(ml_dtypes.bfloat16)

    # V bias folds into the out-proj bias exactly (softmax rows sum to 1):
    # out = Wo @ (A + bv) + bo = Wo @ A + (Wo @ bv + bo)
    bv_sa = inp["sa_in_b"][2 * D:]
    bv_ca = inp["ca_in_b"][2 * D:]
    bo_sa = inp["sa_out_b"] + inp["sa_out_w"] @ bv_sa
    bo_ca = inp["ca_out_b"] + inp["ca_out_w"] @ bv_ca

    shared = {
        "w_sa": bt(inp["sa_in_w"]), "bqk_sa": _col_major(inp["sa_in_b"][:2 * D], 16),
        "wo_sa": bt(16.0 * inp["sa_out_w"]), "bo_sa": _col_major(bo_sa, NC),
        "w_ca": bt(inp["ca_in_w"]), "bqk_ca": _col_major(inp["ca_in_b"][:2 * D], 16),
        "wo_ca": bt(16.0 * inp["ca_out_w"]), "bo_ca": _col_major(bo_ca, NC),
        "w1": bt(inp["ff_w1"]), "b1": _col_major(inp["ff_b1"], NF),
        "w2": bt(inp["ff_w2"]), "b2": _col_major(inp["ff_b2"], NC),
        "ln_w": np.ascontiguousarray(np.stack(
            [inp["ln1_w"], inp["ln2_w"], inp["ln3_w"]]).reshape(3, NC, P)
            .transpose(2, 0, 1)),
        "ln_b": np.ascontiguousarray(np.stack(
            [inp["ln1_b"], inp["ln2_b"], inp["ln3_b"]]).reshape(3, NC, P)
            .transpose(2, 0, 1)),
    }
    perms = []
    in_maps = []
    for c in range(8):
        b, r = c // 2, c % 2
        perm = np.r_[512 * r: 512 * r + 512, 512 * (1 - r): 512 * (1 - r) + 512]
        perms.append(perm)
        in_maps.append({
            "xT16": np.ascontiguousarray(
                inp["x"][b][perm].T).astype(ml_dtypes.bfloat16),
            "xaT16": np.ascontiguousarray(inp["xa"][b].T).astype(ml_dtypes.bfloat16),
            **shared,
        })
    return in_maps, perms


def kernel(**inputs):
    in_maps, perms = build_in_maps(inputs)
    res = run_bass_kernel_spmd(_get_nc(), in_maps, core_ids=list(range(8)))

    x = np.empty((B, L, D), np.float32)
    self_w = np.empty((B, L, L), np.float32)
    cross_w = np.empty((B, L, L), np.float32)
    for c in range(8):
        b, r = c // 2, c % 2
        rows = slice(512 * r, 512 * r + 512)
        x[b, rows] = res.results[c]["xoutT"].T
        # b (int) + perm (array) are both advanced indices separated by a
        # slice, so numpy puts the perm dim first: target shape (1024, 512)
        # with semantics self_w[b, l, perm[j]] = selfwT[j, l].
        self_w[b, rows.start: rows.stop, perms[c]] = res.results[c]["selfwT"]
        cross_w[b, rows] = res.results[c]["crosswT"].T
    return (x, self_w, cross_w)
